# revision 23
# baseline (speedup 1.0000x reference)
"""Deformable self-attention kernel for Trainium2 (8 NeuronCores).

Structural reduction: the sampling offsets are ``tanh(...) * (2/128)`` with
``|tanh| < 1``, added to *integer* grid coordinates and then rounded.  Since
the perturbation magnitude is < 0.5, ``round(c + d) == c`` always, so the
gather indices are exactly ``arange(N)`` (identity), independent of the data.
Each token attends only to itself at all 7 points; the 7 scores are equal, so
softmax is uniform and the attention output equals ``v``.  The whole module
therefore computes

    out = (x @ Wv + bv) @ Wo + bo = x @ (Wv @ Wo) + (bv @ Wo + bo)

This version folds W = Wv @ Wo on the host (cheap: 512^3) and adds the
(usually zero) effective bias on the host, so the device does exactly one
[2048, 512] @ [512, 512] matmul per core, in fp16:

  - x is marshaled host-side to fp16 x^T chunk blocks [P, KT*CW] so every
    DMA moves 2 KB-contiguous per-partition runs;
  - W is fp16 [P, KT*D]; all loads ride the SP HWDGE ring except the first
    two x chunks (Activation ring) so both rings stream during the head;
  - the PE is kept spinning on dummy matmuls during the DMA head so the
    2.4 GHz p-state ramp (~5 us of continuous PE activity) completes before
    the real matmuls arrive;
  - two token tiles accumulate into one 2-bank PSUM tile, drained by a
    single DVE copy (fp32 -> fp16), stored via the Activation HWDGE ring.

HBM traffic per core: 2 MB x + 0.5 MB W + 2 MB out = 4.5 MB (vs 10.5 fp32).
"""

import os
import sys

import numpy as np

for _p in ("/opt/trn_rl_repo", "/root/.axon_site/_ro/trn_rl_repo"):
    if os.path.isdir(_p) and _p not in sys.path:
        sys.path.append(_p)

import concourse.bass as bass  # noqa: F401  (import side effects)
import concourse.mybir as mybir
import concourse.tile as tile
from concourse import bacc
from concourse.bass_utils import run_bass_kernel_spmd

N_CORES = 8
N = 16384          # tokens (128 x 128 grid)
D = 512            # d_model
T = N // N_CORES   # tokens per core (2048)
P = 128            # partitions
KT = D // P        # contraction k-tiles (4)
CW = 256           # tokens per x chunk (= 2 token tiles)
NCH = T // CW      # chunks per core (8)
NWARM = 36         # PE warmup matmuls (p-state ramp) during the DMA head
F32 = mybir.dt.float32
F16 = mybir.dt.float16

_PROGRAM_CACHE = {}


def build_program() -> bacc.Bacc:
    nc = bacc.Bacc("TRN2", target_bir_lowering=False, debug=False)
    xh = [
        nc.dram_tensor(f"xh{c}", [P, KT * CW], F16, kind="ExternalInput").ap()
        for c in range(NCH)
    ]
    wh = nc.dram_tensor("wh", [P, KT * D], F16, kind="ExternalInput").ap()
    oh = nc.dram_tensor("oh", [P, NCH * 2 * D], F16, kind="ExternalOutput").ap()

    with tile.TileContext(nc) as tc:
        with (
            tc.tile_pool(name="consts", bufs=1) as consts,
            tc.tile_pool(name="wpool", bufs=1) as wpool,
            tc.tile_pool(name="xpool", bufs=1) as xpool,
            tc.tile_pool(name="opool", bufs=4) as opool,
            tc.tile_pool(name="po", bufs=6, space="PSUM") as po,
            tc.tile_pool(name="pwarm", bufs=1, space="PSUM") as pwarm,
        ):
            # PE warmup: spin the tensor engine on a dummy [128,128] matmul
            # so the DVFS ramp to 2.4 GHz runs during the DMA head.
            dm = consts.tile([P, P], F16)
            nc.vector.memset(dm, 0.25)
            warm = pwarm.tile([P, P], F32)
            for _ in range(NWARM):
                nc.tensor.matmul(warm, lhsT=dm, rhs=dm, start=True, stop=True)

            # Loads: W alone on the SP ring (it gates every matmul); all x
            # chunks stream on the Activation ring in parallel with W.  A
            # tiny dummy DMA ahead of W absorbs the SDMA engines' cold-start
            # (first-descriptor latency + engine stagger).
            dscr = consts.tile([P, 256], F16)
            nc.sync.dma_start(out=dscr, in_=wh[:, 0:256])
            w_sb = wpool.tile([P, KT, D], F16)
            nc.sync.dma_start(out=w_sb, in_=wh.rearrange("p (k d) -> p k d", k=KT))
            xtr = xpool.tile([P, NCH, KT * CW], F16)
            for c in range(NCH):
                nc.scalar.dma_start(out=xtr[:, c, :], in_=xh[c])

            # Main loop: per chunk, two 128-token tiles accumulate (over 4
            # k-slices) into one 2-bank PSUM tile; single DVE drain casts to
            # fp16; store rides the Activation ring.
            oh_r = oh.rearrange("p (b s d) -> p b s d", b=NCH, s=2)
            for b in range(NCH):
                obuf = opool.tile([P, 2, D], F16, tag="ob", name=f"ob{b}")
                for s in range(2):
                    # per-tile psum + drain authored right after its own
                    # matmul group, so each cast waits only its 4 matmuls
                    pso = po.tile([P, D], F32, tag="pso", name=f"pso{2 * b + s}")
                    for k in range(KT):
                        nc.tensor.matmul(
                            pso,
                            lhsT=xtr[:, b, k * CW + s * P:k * CW + (s + 1) * P],
                            rhs=w_sb[:, k, :],
                            start=(k == 0),
                            stop=(k == KT - 1),
                        )
                    nc.vector.tensor_copy(out=obuf[:, s, :], in_=pso)
                if b < NCH - 1:
                    nc.scalar.dma_start(out=oh_r[:, b], in_=obuf)
                else:
                    # final batch: split the store across both HWDGE rings
                    # (sync ring is idle by now) to shorten the tail chain
                    nc.scalar.dma_start(out=oh_r[:, b, 0], in_=obuf[:, 0, :])
                    nc.sync.dma_start(out=oh_r[:, b, 1], in_=obuf[:, 1, :])
    nc.compile()
    return nc


def _get_program(with_bias: bool = False) -> bacc.Bacc:
    # with_bias kept for test.py compatibility; bias is folded on the host.
    if "p" not in _PROGRAM_CACHE:
        _PROGRAM_CACHE["p"] = build_program()
    return _PROGRAM_CACHE["p"]


def make_in_maps(x, Wv, bv, Wo, bo):
    """Marshal inputs: fold W on host, cast to fp16, chunk-block x^T."""
    x2 = np.asarray(x, dtype=np.float32).reshape(N, D)
    w = (np.asarray(Wv, np.float32) @ np.asarray(Wo, np.float32))
    # wh[p, k*D + d] = W[k*128 + p, d]
    wh = np.ascontiguousarray(
        w.reshape(KT, P, D).transpose(1, 0, 2).reshape(P, KT * D)
    ).astype(np.float16)
    in_maps = []
    for c in range(N_CORES):
        xs = x2[c * T:(c + 1) * T]  # [T, D]
        # xh[ch][p, k*CW + t] = xs[ch*CW + t, k*128 + p]
        xb = (
            xs.reshape(NCH, CW, KT, P)
            .transpose(0, 3, 2, 1)
            .reshape(NCH, P, KT * CW)
            .astype(np.float16)
        )
        m = {f"xh{ch}": np.ascontiguousarray(xb[ch]) for ch in range(NCH)}
        m["wh"] = wh
        in_maps.append(m)
    return in_maps, False


def assemble_output(res, Wo=None, bv=None, bo=None):
    """Unmarshal per-core oh [P, NCH*2*D] fp16 -> [1, N, D] fp32 (+ bias)."""
    parts = []
    for c in range(N_CORES):
        oc = res.results[c]["oh"].reshape(P, NCH * 2, D)
        parts.append(oc.transpose(1, 0, 2).reshape(T, D))
    out = np.concatenate(parts, axis=0).astype(np.float32)
    if Wo is not None:
        beff = (
            np.asarray(bv, np.float32) @ np.asarray(Wo, np.float32)
            + np.asarray(bo, np.float32)
        )
        if np.any(beff):
            out += beff[None, :]
    return out.reshape(1, N, D)


def kernel(x, H, W, Wq, bq, Wk, bk, Wv, bv, Wo, bo, Woff1, boff1, Woff2, boff2,
           **_ignored):
    in_maps, _ = make_in_maps(x, Wv, bv, Wo, bo)
    nc = _get_program()
    res = run_bass_kernel_spmd(nc, in_maps, core_ids=list(range(N_CORES)))
    return assemble_output(res, Wo=Wo, bv=bv, bo=bo)


# revision 26
# speedup vs baseline: 1.0637x; 1.0637x over previous
"""Deformable self-attention kernel for Trainium2 (8 NeuronCores).

Structural reduction: the sampling offsets are ``tanh(...) * (2/128)`` with
``|tanh| < 1``, added to *integer* grid coordinates and then rounded.  Since
the perturbation magnitude is < 0.5, ``round(c + d) == c`` always, so the
gather indices are exactly ``arange(N)`` (identity), independent of the data.
Each token attends only to itself at all 7 points; the 7 scores are equal, so
softmax is uniform and the attention output equals ``v``.  The whole module
therefore computes

    out = (x @ Wv + bv) @ Wo + bo = x @ (Wv @ Wo) + (bv @ Wo + bo)

This version folds W = Wv @ Wo on the host (cheap: 512^3) and adds the
(usually zero) effective bias on the host, so the device does exactly one
[2048, 512] @ [512, 512] matmul per core, in fp16:

  - x is marshaled host-side to fp16 x^T chunk blocks [P, KT*CW] so every
    DMA moves 2 KB-contiguous per-partition runs;
  - W is fp16 [P, KT*D]; all loads ride the SP HWDGE ring except the first
    two x chunks (Activation ring) so both rings stream during the head;
  - the PE is kept spinning on dummy matmuls during the DMA head so the
    2.4 GHz p-state ramp (~5 us of continuous PE activity) completes before
    the real matmuls arrive;
  - two token tiles accumulate into one 2-bank PSUM tile, drained by a
    single DVE copy (fp32 -> fp16), stored via the Activation HWDGE ring.

HBM traffic per core: 2 MB x + 0.5 MB W + 2 MB out = 4.5 MB (vs 10.5 fp32).
"""

import os
import sys

import numpy as np

for _p in ("/opt/trn_rl_repo", "/root/.axon_site/_ro/trn_rl_repo"):
    if os.path.isdir(_p) and _p not in sys.path:
        sys.path.append(_p)

import concourse.bass as bass  # noqa: F401  (import side effects)
import concourse.mybir as mybir
import concourse.tile as tile
from concourse import bacc
from concourse.bass_utils import run_bass_kernel_spmd

N_CORES = 8
N = 16384          # tokens (128 x 128 grid)
D = 512            # d_model
T = N // N_CORES   # tokens per core (2048)
P = 128            # partitions
KT = D // P        # contraction k-tiles (4)
CW = 256           # tokens per x chunk (= 2 token tiles)
NCH = T // CW      # chunks per core (8)
NWARM = 40         # PE warmup matmuls (p-state ramp) during the DMA head
F32 = mybir.dt.float32
F16 = mybir.dt.float16

_PROGRAM_CACHE = {}


def build_program() -> bacc.Bacc:
    nc = bacc.Bacc("TRN2", target_bir_lowering=False, debug=False)
    xh = [
        nc.dram_tensor(f"xh{c}", [P, KT * CW], F16, kind="ExternalInput").ap()
        for c in range(NCH)
    ]
    wh = nc.dram_tensor("wh", [P, KT * D], F16, kind="ExternalInput").ap()
    oh = nc.dram_tensor("oh", [P, NCH * 2 * D], F16, kind="ExternalOutput").ap()

    with tile.TileContext(nc) as tc:
        with (
            tc.tile_pool(name="consts", bufs=1) as consts,
            tc.tile_pool(name="wpool", bufs=1) as wpool,
            tc.tile_pool(name="xpool", bufs=1) as xpool,
            tc.tile_pool(name="opool", bufs=4) as opool,
            tc.tile_pool(name="po", bufs=6, space="PSUM") as po,
            tc.tile_pool(name="pwarm", bufs=1, space="PSUM") as pwarm,
        ):
            # PE warmup: spin the tensor engine on a dummy [128,128] matmul
            # so the DVFS ramp to 2.4 GHz runs during the DMA head.
            dm = consts.tile([P, P], F16)
            nc.vector.memset(dm, 0.25)
            warm = pwarm.tile([P, P], F32)
            for _ in range(NWARM):
                nc.tensor.matmul(warm, lhsT=dm, rhs=dm, start=True, stop=True)

            # Loads: W alone on the SP ring (it gates every matmul); all x
            # chunks stream on the Activation ring in parallel with W.
            w_sb = wpool.tile([P, KT, D], F16)
            nc.sync.dma_start(out=w_sb, in_=wh.rearrange("p (k d) -> p k d", k=KT))
            xtr = xpool.tile([P, NCH, KT * CW], F16)
            for c in range(NCH):
                nc.scalar.dma_start(out=xtr[:, c, :], in_=xh[c])

            # Main loop: per chunk, two 128-token tiles accumulate (over 4
            # k-slices) into one 2-bank PSUM tile; single DVE drain casts to
            # fp16; store rides the Activation ring.
            oh_r = oh.rearrange("p (b s d) -> p b s d", b=NCH, s=2)
            for b in range(NCH):
                obuf = opool.tile([P, 2, D], F16, tag="ob", name=f"ob{b}")
                for s in range(2):
                    # per-tile psum + drain authored right after its own
                    # matmul group, so each cast waits only its 4 matmuls
                    pso = po.tile([P, D], F32, tag="pso", name=f"pso{2 * b + s}")
                    for k in range(KT):
                        nc.tensor.matmul(
                            pso,
                            lhsT=xtr[:, b, k * CW + s * P:k * CW + (s + 1) * P],
                            rhs=w_sb[:, k, :],
                            start=(k == 0),
                            stop=(k == KT - 1),
                        )
                    nc.vector.tensor_copy(out=obuf[:, s, :], in_=pso)
                if b < NCH - 1:
                    # odd batches ride the otherwise-idle SP ring: spreads
                    # store traffic and keeps that ring's DGE warm for the
                    # final tail store
                    eng = nc.sync if b % 2 == 1 else nc.scalar
                    eng.dma_start(out=oh_r[:, b], in_=obuf)
                else:
                    # final batch: split the store across both HWDGE rings
                    # (sync ring is idle by now) to shorten the tail chain
                    nc.scalar.dma_start(out=oh_r[:, b, 0], in_=obuf[:, 0, :])
                    nc.sync.dma_start(out=oh_r[:, b, 1], in_=obuf[:, 1, :])
    nc.compile()
    return nc


def _get_program(with_bias: bool = False) -> bacc.Bacc:
    # with_bias kept for test.py compatibility; bias is folded on the host.
    if "p" not in _PROGRAM_CACHE:
        _PROGRAM_CACHE["p"] = build_program()
    return _PROGRAM_CACHE["p"]


def make_in_maps(x, Wv, bv, Wo, bo):
    """Marshal inputs: fold W on host, cast to fp16, chunk-block x^T."""
    x2 = np.asarray(x, dtype=np.float32).reshape(N, D)
    w = (np.asarray(Wv, np.float32) @ np.asarray(Wo, np.float32))
    # wh[p, k*D + d] = W[k*128 + p, d]
    wh = np.ascontiguousarray(
        w.reshape(KT, P, D).transpose(1, 0, 2).reshape(P, KT * D)
    ).astype(np.float16)
    in_maps = []
    for c in range(N_CORES):
        xs = x2[c * T:(c + 1) * T]  # [T, D]
        # xh[ch][p, k*CW + t] = xs[ch*CW + t, k*128 + p]
        xb = (
            xs.reshape(NCH, CW, KT, P)
            .transpose(0, 3, 2, 1)
            .reshape(NCH, P, KT * CW)
            .astype(np.float16)
        )
        m = {f"xh{ch}": np.ascontiguousarray(xb[ch]) for ch in range(NCH)}
        m["wh"] = wh
        in_maps.append(m)
    return in_maps, False


def assemble_output(res, Wo=None, bv=None, bo=None):
    """Unmarshal per-core oh [P, NCH*2*D] fp16 -> [1, N, D] fp32 (+ bias)."""
    parts = []
    for c in range(N_CORES):
        oc = res.results[c]["oh"].reshape(P, NCH * 2, D)
        parts.append(oc.transpose(1, 0, 2).reshape(T, D))
    out = np.concatenate(parts, axis=0).astype(np.float32)
    if Wo is not None:
        beff = (
            np.asarray(bv, np.float32) @ np.asarray(Wo, np.float32)
            + np.asarray(bo, np.float32)
        )
        if np.any(beff):
            out += beff[None, :]
    return out.reshape(1, N, D)


def kernel(x, H, W, Wq, bq, Wk, bk, Wv, bv, Wo, bo, Woff1, boff1, Woff2, boff2,
           **_ignored):
    in_maps, _ = make_in_maps(x, Wv, bv, Wo, bo)
    nc = _get_program()
    res = run_bass_kernel_spmd(nc, in_maps, core_ids=list(range(N_CORES)))
    return assemble_output(res, Wo=Wo, bv=bv, bo=bo)
